# revision 1
# baseline (speedup 1.0000x reference)
"""Trainium2 Bass kernel for GQA MultiHeadAttention + LoRA + ALiBi + causal mask.

Problem (hardcoded): B=2, S=2048, D=1024, H=16 q-heads, KVH=4 kv-heads, DK=64,
LoRA rank 16, ALiBi alpha 1.0, causal, softmax, eval mode.

Sharding over 8 cores: core c = 4*b + g handles batch b and kv-group g
(query heads 4g..4g+3, kv head g).

Host precompute:
  - LoRA folded into weights:  W_eff = W + A@B (SCALING=1).
  - Score scale (1/sqrt(64)) folded into W_eff_q.
  - Inputs passed pre-transposed (xT = x.T) because the PE contracts over the
    partition dim.
  - ALiBi folded into the score matmul as 2 extra contraction rows:
      kext = [k_idx; 1],  qext_h = [slope_h; -slope_h * q_idx]
    so S'[k,q] = scale*q.k + slope_h*(k - q) comes out of one matmul (K=66).

Device dataflow (per core, all matmul operands float32r = full-rate fp32):
  1. Projections into transposed layouts: Q^T (per head, 66 rows with ext),
     K^T (66 rows with ext), V natural [k,64] + ones column (row sums ride
     along the attention matmul: O^T_ext row 64 = softmax denominator).
  2. Per q-block of 512, per k-tile of 128: S^T = K^T_ext.T @ Q^T_ext on PE
     (contraction 66, [k, q] layout), exp on ACT over 2-head groups,
     causal diagonal handled by narrowing the q-span + one additive
     [128,128] mask (0 / -1e30) before exp, O^T accumulated in PSUM.
  3. Normalize: reciprocal of denominator row (DVE), broadcast across
     partitions (GpSimd), multiply (DVE).
  4. Output projection computed transposed (Y^T), its 8 column-tile units
     drip-fed into the next q-block's PE stream; host sums the 4 partial
     Y^T per batch and transposes.

  Schedule: proj(sb) / attention(sb-1) / outproj(sb-1) software-pipelined;
  PSUM budget 2 (proj+outproj) + 4 (scores, double-buffered 2-head groups)
  + 2 (O^T accumulators, head-pair passes) = 8 banks.
"""

import sys

sys.path.insert(0, "/opt/trn_rl_repo")

import numpy as np

import concourse.bass as bass  # noqa: F401  (bass must import before bacc)
import concourse.mybir as mybir
from concourse import bacc
from concourse.bass_utils import run_bass_kernel_spmd
from concourse.tile import TileContext

F32R = mybir.dt.float32r
F32 = mybir.dt.float32
EXP = mybir.ActivationFunctionType.Exp

D = 1024
DK = 64
NHL = 4  # query heads per core
NCORES = 8

LAST_EXEC_NS = None
_NC_CACHE = {}


def build_nc(S):
    NQB = S // 512  # q-blocks of 512
    NKT = S // 128  # k-tiles of 128
    NSB = S // 512  # seq blocks of 512 (projections)

    nc = bacc.Bacc()
    qT = nc.declare_dram_parameter("qT", [D, S], F32R, isOutput=False)
    kT = nc.declare_dram_parameter("kT", [D, S], F32R, isOutput=False)
    vT = nc.declare_dram_parameter("vT", [D, S], F32R, isOutput=False)
    wq = nc.declare_dram_parameter("wq", [128, 8, 256], F32R, isOutput=False)
    wk = nc.declare_dram_parameter("wk", [128, 8, 64], F32R, isOutput=False)
    wv = nc.declare_dram_parameter("wv", [128, 8, 64], F32R, isOutput=False)
    wo = nc.declare_dram_parameter("wo", [128, 2, D], F32R, isOutput=False)
    qext = nc.declare_dram_parameter("qext", [2, NHL, S], F32R, isOutput=False)
    kext = nc.declare_dram_parameter("kext", [2, S], F32R, isOutput=False)
    dmask = nc.declare_dram_parameter("dmask", [128, 2, 128], F32, isOutput=False)
    vones = nc.declare_dram_parameter("vones", [128, NKT, 1], F32R, isOutput=False)
    ident = nc.declare_dram_parameter("ident", [64, 64], F32R, isOutput=False)
    yT = nc.declare_dram_parameter("yT", [D, S], F32, isOutput=True)

    with TileContext(nc) as tc:
        with (
            tc.sbuf_pool(name="cst", bufs=1) as cp,
            tc.sbuf_pool(name="xin", bufs=16) as xp,
            tc.sbuf_pool(name="stg", bufs=3) as sp,
            tc.sbuf_pool(name="pbuf", bufs=4) as pbp,
            tc.sbuf_pool(name="rbuf", bufs=6) as rp,
            tc.sbuf_pool(name="yout", bufs=4) as yp_sb,
            tc.psum_pool(name="pj", bufs=2) as pj,    # 2 banks: proj + outproj
            tc.psum_pool(name="sc", bufs=2) as scp,   # 4 banks: scores (double buffered)
            tc.psum_pool(name="ob", bufs=2) as obp,   # 2 banks: O^T accum (head pair)
        ):
            # ---- resident weights / constants (ACT hwdge queue; host pre-layouted) ----
            # attention-gating consts first on the fast ACT queue, in
            # criticality order (kext/qext feed the first score matmuls,
            # vones the first O^T accumulation)
            qt_all = cp.tile([66, NHL, S], F32R)
            kt_sb = cp.tile([66, S], F32R)
            vext = cp.tile([128, NKT, 65], F32R)
            nc.scalar.dma_start(out=kt_sb[64:66, :], in_=kext[:])
            nc.scalar.dma_start(out=qt_all[64:66, :, :], in_=qext[:])
            nc.scalar.dma_start(out=vext[:, :, 64:65], in_=vones[:])

            wq_sb = cp.tile([128, 8, 256], F32R)
            nc.gpsimd.dma_start(out=wq_sb[:], in_=wq[:])
            wk_sb = cp.tile([128, 8, 64], F32R)
            nc.gpsimd.dma_start(out=wk_sb[:], in_=wk[:])
            wv_sb = cp.tile([128, 8, 64], F32R)
            nc.gpsimd.dma_start(out=wv_sb[:], in_=wv[:])
            ident_sb = cp.tile([64, 64], F32R)
            nc.scalar.dma_start(out=ident_sb[:], in_=ident[:])
            dmask_sb = cp.tile([128, 2, 128], F32)
            nc.scalar.dma_start(out=dmask_sb[:], in_=dmask[:])
            wo_sb = cp.tile([128, 2, D], F32R)
            nc.gpsimd.dma_start(out=wo_sb[:], in_=wo[:])

            otf = [cp.tile([128, S], F32R, name=f"otf{ch}") for ch in range(2)]

            def proj_block(sb):
                """Q/K/V projections for seq block sb (columns 512*sb..+512)."""
                cols = slice(512 * sb, 512 * (sb + 1))
                # stream x^T chunks for all three inputs on the SP queue
                xq, xk, xv = [], [], []
                for nm, src, lst in (("q", qT, xq), ("k", kT, xk), ("v", vT, xv)):
                    for ci in range(8):
                        t = xp.tile([128, 512], F32R, name=f"x{nm}{sb}_{ci}", tag="x")
                        nc.sync.dma_start(
                            out=t[:], in_=src[128 * ci : 128 * (ci + 1), cols]
                        )
                        lst.append(t)
                # Q: two 128-row (2-head) output tiles; odd heads staged and
                # partition-shifted per M-tile (keeps pair-0's head 1 off
                # pair-1's dependency chain)
                for mt in range(2):
                    ps = pj.tile([128, 512], F32, name=f"qp{sb}_{mt}", tag="pj")
                    for ci in range(8):
                        nc.tensor.matmul(
                            ps[:],
                            lhsT=wq_sb[:, ci, 128 * mt : 128 * (mt + 1)],
                            rhs=xq[ci][:],
                            start=(ci == 0),
                            stop=(ci == 7),
                        )
                    heven, hodd = 2 * mt, 2 * mt + 1
                    # odd-head path (copy + shift DMA) is the longer leg of the
                    # exp dependency — emit it first
                    stg = sp.tile([128, 512], F32R, name=f"qs{sb}_{mt}", tag="st")
                    nc.vector.tensor_copy(stg[64:128, :], ps[64:128, :])
                    nc.scalar.dma_start(
                        out=qt_all[0:64, hodd, cols], in_=stg[64:128, :]
                    )
                    nc.vector.tensor_copy(qt_all[0:64, heven, cols], ps[0:64, :])
                # K
                ps = pj.tile([64, 512], F32, name=f"kp{sb}", tag="pj")
                for ci in range(8):
                    nc.tensor.matmul(
                        ps[:],
                        lhsT=wk_sb[:, ci, :],
                        rhs=xk[ci][:],
                        start=(ci == 0),
                        stop=(ci == 7),
                    )
                nc.vector.tensor_copy(kt_sb[0:64, cols], ps[:])
                # V^T at full matmul rate, then PE-transpose into V layout
                ps = pj.tile([64, 512], F32, name=f"vtp{sb}", tag="pj")
                for ci in range(8):
                    nc.tensor.matmul(
                        ps[:],
                        lhsT=wv_sb[:, ci, :],
                        rhs=xv[ci][:],
                        start=(ci == 0),
                        stop=(ci == 7),
                    )
                vts = sp.tile([64, 512], F32R, name=f"vts{sb}", tag="vt")
                nc.vector.tensor_copy(vts[:], ps[:])
                for sub in range(4):
                    st_ = 4 * sb + sub
                    tp = pj.tile([128, 64], F32R, name=f"vtr{st_}", tag="pj")
                    nc.tensor.transpose(
                        tp[:], vts[:, 128 * sub : 128 * (sub + 1)], ident_sb[:]
                    )
                    nc.vector.tensor_copy(vext[:, st_, 0:64], tp[:])

            def attn_block(qb, pending=None):
                """Attention for q-block qb, head pairs sequentially (2 OT banks).
                One pending outproj unit is emitted per ki iteration so PE
                interleaves them without starving the exp stream."""
                pending = list(pending or [])
                nk = 4 * qb + 4
                dst_cols = slice(512 * qb, 512 * (qb + 1))
                for pr in range(2):
                    ots = [
                        obp.tile([128, 512], F32, name=f"ot{qb}_{pr}_{hh}", tag="ot")
                        for hh in range(2)
                    ]
                    for ki in range(nk):
                        k0 = 128 * ki
                        diag = ki >= 4 * qb
                        if diag:
                            qs, w = k0, 512 - (k0 - 512 * qb)
                        else:
                            qs, w = 512 * qb, 512
                        qoff = qs - 512 * qb
                        scat = scp.tile(
                            [128, 2, 512], F32, name=f"s{qb}_{pr}_{ki}", tag="s"
                        )
                        for hh in range(2):
                            h = 2 * pr + hh
                            nc.tensor.matmul(
                                scat[:, hh, 0:w],
                                lhsT=kt_sb[:, k0 : k0 + 128],
                                rhs=qt_all[:, h, qs : qs + w],
                                start=True,
                                stop=True,
                            )
                        if diag:
                            nc.vector.tensor_add(
                                scat[:, :, 0:128], scat[:, :, 0:128], dmask_sb[:]
                            )
                        p = pbp.tile(
                            [128, 2, 512], F32R, name=f"p{qb}_{pr}_{ki}", tag="p"
                        )
                        nc.scalar.activation(p[:, :, 0:w], scat[:, :, 0:w], EXP)
                        for hh in range(2):
                            nc.tensor.matmul(
                                ots[hh][0:65, qoff : qoff + w],
                                lhsT=vext[:, ki, :],
                                rhs=p[:, hh, 0:w],
                                start=(ki == 0),
                                stop=(ki == nk - 1),
                            )
                        if pending and ki % 2 == 1:
                            pending.pop(0)()
                    # normalize the pair; copy PSUM->SBUF first so the OT
                    # banks free without waiting on the recip/bcast/mul chain
                    for hh in range(2):
                        h = 2 * pr + hh
                        raw = rp.tile([65, 512], F32, name=f"raw{qb}_{h}", tag="raw")
                        nc.vector.tensor_copy(raw[:], ots[hh][0:65, :])
                        rc = rp.tile([1, 512], F32, name=f"rc{qb}_{h}", tag="rc")
                        nc.vector.reciprocal(rc[:], raw[64:65, :])
                        rb = rp.tile([64, 512], F32, name=f"rb{qb}_{h}", tag="rb")
                        nc.gpsimd.partition_broadcast(rb[:], rc[:])
                        ch, half = divmod(h, 2)
                        if half == 0:
                            nc.vector.tensor_mul(
                                otf[ch][0:64, dst_cols], raw[0:64, :], rb[:]
                            )
                        else:
                            s2 = rp.tile([64, 512], F32R, name=f"os{qb}_{h}", tag="os")
                            nc.vector.tensor_mul(s2[:], raw[0:64, :], rb[:])
                            nc.gpsimd.dma_start(
                                out=otf[ch][64:128, dst_cols], in_=s2[:]
                            )
                for u in pending:
                    u()

            def outproj_units(sb, act_copy=False):
                """Y^T columns for seq block sb as 8 schedulable units."""
                cols = slice(512 * sb, 512 * (sb + 1))

                def unit(yt):
                    def run():
                        ps = pj.tile([128, 512], F32, name=f"y{yt}_{sb}", tag="pj")
                        for ch in range(2):
                            nc.tensor.matmul(
                                ps[:],
                                lhsT=wo_sb[:, ch, 128 * yt : 128 * (yt + 1)],
                                rhs=otf[ch][:, cols],
                                start=(ch == 0),
                                stop=(ch == 1),
                            )
                        yo = yp_sb.tile(
                            [128, 512], F32, name=f"yo{yt}_{sb}", tag="yo"
                        )
                        if act_copy:
                            nc.scalar.copy(yo[:], ps[:])
                        else:
                            nc.vector.tensor_copy(yo[:], ps[:])
                        nc.gpsimd.dma_start(
                            out=yT[128 * yt : 128 * (yt + 1), cols], in_=yo[:]
                        )

                    return run

                return [unit(yt) for yt in range(8)]

            # interleaved schedule: proj(sb) feeds attn(qb=sb); outproj(qb)
            # units are drip-fed into the NEXT attention block's PE stream
            pending = []
            proj_block(0)
            for sb in range(1, NSB):
                attn_block(sb - 1, pending)
                pending = outproj_units(sb - 1)
                proj_block(sb)
            attn_block(NQB - 1, pending)
            for u in outproj_units(NQB - 1, act_copy=True):
                u()

    nc.compile()
    return nc


def _get_nc(S):
    if S not in _NC_CACHE:
        _NC_CACHE[S] = build_nc(S)
    return _NC_CACHE[S]


def kernel(**inputs):
    global LAST_EXEC_NS
    f = np.float32
    query = np.asarray(inputs["query"], f)
    key = np.asarray(inputs["key"], f)
    value = np.asarray(inputs["value"], f)
    B, S, _ = query.shape

    scale = f(1.0 / np.sqrt(DK))
    Wq = np.asarray(inputs["Wq"], f) + np.asarray(inputs["Aq"], f) @ np.asarray(inputs["Bq"], f)
    Wq = Wq * scale
    Wk = np.asarray(inputs["Wk"], f) + np.asarray(inputs["Ak"], f) @ np.asarray(inputs["Bk"], f)
    Wv = np.asarray(inputs["Wv"], f) + np.asarray(inputs["Av"], f) @ np.asarray(inputs["Bv"], f)
    Wo = np.asarray(inputs["Wo"], f) + np.asarray(inputs["Ao"], f) @ np.asarray(inputs["Bo"], f)

    k_idx = np.arange(S, dtype=f)
    k_cent = k_idx - f(S // 2)  # centered: exactly representable after f32r rounding
    kext = np.stack([k_cent, np.ones(S, f)])  # [2, S]
    # additive causal mask for the 128x128 diagonal block: 0 keep, -1e30 drop
    # (replicated for the 2-head score groups: [128, 2, 128])
    dm1 = np.where(np.triu(np.ones((128, 128), bool)), f(0), f(-1e30))
    dmask = np.ascontiguousarray(np.stack([dm1, dm1], axis=1))
    vones = np.ones((128, S // 128, 1), f)
    ident = np.eye(64, dtype=f)

    in_maps = []
    for c in range(NCORES):
        b, g = divmod(c, 4)
        qe = np.empty((2, NHL, S), f)
        for hl in range(NHL):
            slope = f(2.0 ** (-(4 * g + hl + 1)))
            qe[0, hl] = slope
            qe[1, hl] = -slope * k_cent
        in_maps.append(
            {
                "qT": np.ascontiguousarray(query[b].T),
                "kT": np.ascontiguousarray(key[b].T),
                "vT": np.ascontiguousarray(value[b].T),
                "wq": np.ascontiguousarray(
                    Wq[:, 256 * g : 256 * (g + 1)].reshape(8, 128, 256).transpose(1, 0, 2)
                ),
                "wk": np.ascontiguousarray(
                    Wk[:, 64 * g : 64 * (g + 1)].reshape(8, 128, 64).transpose(1, 0, 2)
                ),
                "wv": np.ascontiguousarray(
                    Wv[:, 64 * g : 64 * (g + 1)].reshape(8, 128, 64).transpose(1, 0, 2)
                ),
                "wo": np.ascontiguousarray(
                    Wo[256 * g : 256 * (g + 1), :].reshape(2, 128, D).transpose(1, 0, 2)
                ),
                "qext": qe,
                "kext": kext,
                "dmask": dmask,
                "vones": vones,
                "ident": ident,
            }
        )

    nc = _get_nc(S)
    res = run_bass_kernel_spmd(nc, in_maps, list(range(NCORES)))
    LAST_EXEC_NS = res.exec_time_ns

    out = np.empty((B, S, D), f)
    for b in range(B):
        acc = res.results[4 * b + 0]["yT"].astype(f)
        for g in range(1, 4):
            acc = acc + res.results[4 * b + g]["yT"]
        out[b] = acc.T
    return out

